# revision 26
# baseline (speedup 1.0000x reference)
"""Trainium2 Bass kernel for the Aligator smoothing-filter problem.

Math notes (all derivable from the reference):
  * delta = max-min of each series, and the EMA level always stays inside
    [min, max], so the clip in the reference never binds -> each per-sigma
    filter is the pure linear recurrence new_t = (1-s)*new_{t-1} + s*y_t
    (new_{-1} = y_0), filt_t = new_{t-1} (filt_0 = y_0).
  * The innovation r_t = y_t - new_{t-1} obeys r_{t+1} = (1-s)*r_t + D_t with
    D_t = y_{t+1} - y_t shared by ALL sigmas, r_1 = D_0.  So per sigma we need
    one affine scan over D plus one square-accumulate:
        err_sum = sum_{t>=1} r_t^2     (mean = /T; the t=0 term is exactly 0)
        final   = new_{T-1} = y_{T-1} - (1-s)*r_{T-1}
  * The batch-carryover argmin is a tiny [B,7] running-min scan -> host.
  * sm[b] = filt of the last "improvement" row <= b.  There are only K (~10)
    unique improvement rows; reconstruct their filts with one device scan and
    gather/broadcast them to all 2048 output rows with a one-hot fp32 matmul
    on the TensorEngine, then out = data - sm on the VectorEngine.

Two SPMD NEFFs over 8 cores (batch-sharded 256 rows/core), with the cheap
selection scan on host between them.

Scheduling discipline: this toolchain caps each compute instruction at ONE
sync wait.  Same-engine (drain) waits merge into one, so every instruction
is arranged to have at most one *cross-engine/DMA* dependency that is not
already covered by its engine's vector clock; tiny "touch" ops absorb the
rest ahead of time.
"""

import numpy as np

import concourse.bass as bass
import concourse.mybir as mybir
import concourse.tile as tile
from concourse.bass_utils import run_bass_kernel_spmd

F32 = mybir.dt.float32
AF = mybir.ActivationFunctionType
OP = mybir.AluOpType

LRS = np.array([0.01, 0.08, 0.1, 0.15, 0.2, 0.25, 1.0], dtype=np.float32)
NLR = 7
PRED_SIZE = 64
B, T = 2048, 4096
NCORES = 8
RPC = B // NCORES          # rows per core = 256
RT = RPC // 128            # row-tiles per core = 2
KPAD = 128                 # padded count of improvement rows
CH = 1024                  # scan chunk width (constant-tile width)

# Optionally filled with BassKernelResults by run (test.py reads these).
LAST_RESULTS = []
TRACE = False


def _chunks(n, ch):
    c0 = 0
    while c0 < n:
        yield c0, min(c0 + ch, n)
        c0 += ch


def _legalize_waits(nc, cap_evsem=2):
    """This walrus build caps sync waits at 1 per compute/ctrl instruction
    (2 for EventSemaphore).  Hoist excess waits onto standalone
    EventSemaphore instructions injected just before the offender on the
    same engine queue."""
    import bass_rust

    for fn in nc.m.functions:
        for blk in fn.blocks:
            newinsts = []
            for I in blk.instructions:
                si = I.sync_info
                waits = list(si.on_wait) if (si and si.on_wait) else []
                cap = cap_evsem if isinstance(I, mybir.InstEventSemaphore) else 1
                if len(waits) > cap:
                    keep = waits[-cap:]
                    excess = waits[:-cap]
                    for i in range(0, len(excess), cap_evsem):
                        chunk = excess[i : i + cap_evsem]
                        ev = mybir.InstEventSemaphore(
                            name=f"{I.name}-wsplit-{i}", ins=[], outs=[]
                        )
                        ev.engine = I.engine
                        ev.sync_info = bass_rust.SyncInfo(
                            on_wait=chunk, on_update=[]
                        )
                        newinsts.append(ev)
                    I.sync_info = bass_rust.SyncInfo(
                        on_wait=keep,
                        on_update=list(si.on_update) if si.on_update else [],
                    )
                newinsts.append(I)
            blk.instructions[:] = newinsts
    return nc


def _after(inst, *preds):
    """Pin scheduler ordering (no semaphore): inst must follow preds."""
    from concourse.instruction_name_ordered_set import InstructionNameOrderedSet

    deps = InstructionNameOrderedSet()
    for p in preds:
        deps.add(p.ins.name)
    inst.ins.add_nosync_dependencies_from(deps)
    return inst


# --------------------------------------------------------------------------
# Phase-1 program: errs + finals per (row, sigma)
# --------------------------------------------------------------------------

def build_phase1():
    nc = bass.Bass()
    data = nc.declare_dram_parameter("data", [RT, 128, T], F32, isOutput=False)
    errs_o = nc.declare_dram_parameter("errs", [RT, 128, NLR], F32, isOutput=True)
    finals_o = nc.declare_dram_parameter("finals", [RT, 128, NLR], F32, isOutput=True)

    with tile.TileContext(nc) as tc:
        with (
            tc.tile_pool(name="io", bufs=2) as io,
            tc.tile_pool(name="dl", bufs=2) as dl,
            tc.tile_pool(name="atp", bufs=6) as atp,
            tc.tile_pool(name="rtp", bufs=2) as rtp,
            tc.tile_pool(name="scr", bufs=1) as scr,
            tc.tile_pool(name="tch", bufs=16) as tch,
            tc.tile_pool(name="small", bufs=2) as small,
        ):
            # constant (1-sigma) tiles, written once on DVE (fresh: no waits)
            ats = []
            for si in range(6):
                a_t = atp.tile([128, CH], F32, tag="at")
                nc.vector.memset(a_t[:], float(1.0 - LRS[si]))
                ats.append(a_t)

            erfs = []
            sq_insts = {}  # alloc index -> (erf column AP, square inst)
            HM = T // 2  # 2048; sub0 needs one extra column of y
            for r in range(RT):
                y = io.tile([128, T], F32, tag="y")
                # halved loads so the diffs (and first scans) start early
                nc.sync.dma_start(out=y[:, 0 : HM + 1], in_=data[r, :, 0 : HM + 1])
                nc.sync.dma_start(out=y[:, HM + 1 : T], in_=data[r, :, HM + 1 : T])
                # GpSimd: first differences, half by half (off the DVE)
                dlt = dl.tile([128, T - 1], F32, tag="dlt")
                isub0 = nc.gpsimd.tensor_sub(
                    dlt[:, 0:HM], y[:, 1 : HM + 1], y[:, 0:HM]
                )
                isub = _after(
                    nc.gpsimd.tensor_sub(
                        dlt[:, HM : T - 1], y[:, HM + 1 : T], y[:, HM : T - 1]
                    ),
                    isub0,
                )
                # DVE absorbers: observe the Pool drains per half
                tt = tch.tile([128, 1], F32, tag="t")
                vdlt = _after(nc.vector.tensor_copy(tt[:], dlt[:, 0:1]), isub0)
                tt = tch.tile([128, 1], F32, tag="t")
                vdlt1 = _after(
                    nc.vector.tensor_copy(tt[:], dlt[:, HM : HM + 1]), isub, vdlt
                )
                # ACT: waits on the y DMA only
                ylast = small.tile([128, 1], F32, tag="ylast")
                iyl = nc.scalar.copy(ylast[:], y[:, T - 1 : T])
                erf = small.tile([128, NLR], F32, tag="erf")
                fin = small.tile([128, NLR], F32, tag="fin")
                erfs.append(erf)
                # ACT: absorbs the ylast ACT-drain; sigma=1 final = y_{T-1}
                _after(nc.scalar.copy(fin[:, 6:7], ylast[:]), iyl)

                for si in range(6):
                    a = float(1.0 - LRS[si])
                    i_alloc = r * 6 + si
                    rt_ = rtp.tile([128, T - 1], F32, tag="rt")
                    vabs = None
                    if i_alloc >= 2:
                        # DVE absorber: wait out the ACT square that last
                        # read the slot this rt_ recycles
                        ecol, prev_sq = sq_insts[i_alloc - 2]
                        tt = tch.tile([128, 1], F32, tag="t")
                        vabs = _after(nc.vector.tensor_copy(tt[:], ecol), prev_sq)
                    # DVE first-touch of the slot (same-engine WAW only now)
                    im = _after(nc.vector.memset(rt_[:, 0:1], 0.0), vdlt)
                    if vabs is not None:
                        _after(im, vabs)
                    # chunked scan: r_{t+1} = a*r_t + D_t (all deps DVE-local;
                    # chunks beyond HM additionally follow the half-1 absorber)
                    for c0, c1 in _chunks(T - 1, CH):
                        init = 0.0 if c0 == 0 else rt_[:, c0 - 1 : c0]
                        isc = nc.vector.tensor_tensor_scan(
                            rt_[:, c0:c1],
                            ats[si][:, 0 : c1 - c0],
                            dlt[:, c0:c1],
                            init,
                            OP.mult,
                            OP.add,
                        )
                        if c1 > HM:
                            _after(isc, vdlt1)
                    # ACT: final first (absorbs rt_'s DVE drain; 1 wait)
                    ifin = nc.scalar.activation(
                        fin[:, si : si + 1],
                        rt_[:, T - 2 : T - 1],
                        AF.Identity,
                        bias=ylast[:],
                        scale=-a,
                    )
                    # ACT: square-accumulate (rt_ covered; 1 wait on sq WAW)
                    sq = scr.tile([128, T - 1], F32, tag="sq")
                    isq = _after(
                        nc.scalar.activation(
                            sq[:], rt_[:], AF.Square,
                            accum_out=erf[:, si : si + 1],
                        ),
                        ifin,
                    )
                    sq_insts[i_alloc] = (erf[:, si : si + 1], isq)

                # sigma=1.0: r-sequence is dlt itself (now Pool-written);
                # ACT absorber observes the (second) Pool drain first
                tt = tch.tile([128, 1], F32, tag="t")
                aabs = _after(nc.scalar.copy(tt[:], dlt[:, HM : HM + 1]), isub)
                sq = scr.tile([128, T - 1], F32, tag="sq")
                _after(
                    nc.scalar.activation(
                        sq[:], dlt[:], AF.Square, accum_out=erf[:, 6:7]
                    ),
                    aabs,
                )

                nc.sync.dma_start(out=errs_o[r], in_=erf[:])
                nc.sync.dma_start(out=finals_o[r], in_=fin[:])
    return nc


# --------------------------------------------------------------------------
# Phase-2 program: reconstruct sm rows and subtract
# --------------------------------------------------------------------------

def build_phase2():
    nc = bass.Bass()
    data = nc.declare_dram_parameter("data", [RT, 128, T], F32, isOutput=False)
    yk = nc.declare_dram_parameter("yk", [KPAD, T], F32, isOutput=False)
    ak = nc.declare_dram_parameter("ak", [KPAD, 1], F32, isOutput=False)
    sk = nc.declare_dram_parameter("sk", [KPAD, 1], F32, isOutput=False)
    st = nc.declare_dram_parameter("st", [KPAD, RPC], F32, isOutput=False)
    out_o = nc.declare_dram_parameter("out", [RT, 128, T], F32, isOutput=True)

    with tile.TileContext(nc) as tc:
        with (
            tc.tile_pool(name="cons", bufs=1) as cons,
            tc.tile_pool(name="tch", bufs=24) as tch,
            tc.tile_pool(name="io", bufs=2) as io,
            tc.tile_pool(name="psum", bufs=1, space="PSUM") as psum,
        ):
            ykt = cons.tile([KPAD, T], F32)
            akt = cons.tile([KPAD, 1], F32)
            skt = cons.tile([KPAD, 1], F32)
            stt = cons.tile([KPAD, RPC], F32)
            nc.sync.dma_start(out=ykt[:], in_=yk[:])
            nc.sync.dma_start(out=akt[:], in_=ak[:])
            nc.sync.dma_start(out=skt[:], in_=sk[:])
            nc.sync.dma_start(out=stt[:], in_=st[:])

            # ACT chain: touch skt (DMA wait), then d1 = s_k * y_k (ykt DMA)
            t = tch.tile([KPAD, 1], F32, tag="t")
            ia = nc.scalar.copy(t[:], skt[:])
            d1 = cons.tile([KPAD, T], F32)
            id1 = _after(
                nc.scalar.activation(d1[:], ykt[:], AF.Copy, scale=skt[:]), ia
            )

            # DVE touch chain: akt DMA, ykt DMA, d1 ACT drain
            t = tch.tile([KPAD, 1], F32, tag="t")
            v1 = nc.vector.tensor_copy(t[:], akt[:])
            t = tch.tile([KPAD, 1], F32, tag="t")
            v2 = _after(nc.vector.tensor_copy(t[:], ykt[:, 0:1]), v1)
            t = tch.tile([KPAD, 1], F32, tag="t")
            v3 = _after(nc.vector.tensor_copy(t[:], d1[:, 0:1]), v2)
            # ab[p, :] = a_k[p]  (= ykt*0 + akt; both inputs covered above)
            ab = cons.tile([KPAD, CH], F32)
            iab = _after(
                nc.vector.tensor_scalar(
                    ab[:], ykt[:, 0:CH], 0.0, akt[:], OP.mult, OP.add
                ),
                v3,
            )

            # new_t = a*new_{t-1} + s*y_t, init y_0 (chunk-chained, DVE-local)
            new = cons.tile([KPAD, T], F32)
            last_scan = iab
            for c0, c1 in _chunks(T, CH):
                init = ykt[:, 0:1] if c0 == 0 else new[:, c0 - 1 : c0]
                last_scan = _after(
                    nc.vector.tensor_tensor_scan(
                        new[:, c0:c1], ab[:, 0 : c1 - c0], d1[:, c0:c1],
                        init, OP.mult, OP.add,
                    ),
                    last_scan,
                )

            # f32r copies on ACT: the PE streams f32r moving columns at
            # full rate (1 cyc/col vs 4 for plain fp32); ~1.5e-4 rounding.
            F32R = mybir.dt.float32r
            newr = cons.tile([KPAD, T], F32R)
            icnew = _after(nc.scalar.copy(newr[:], new[:]), last_scan)
            sttr = cons.tile([KPAD, RPC], F32R)
            icstt = _after(nc.scalar.copy(sttr[:], stt[:]), icnew)
            ykr = cons.tile([KPAD, 1], F32R)
            iyk = _after(nc.scalar.copy(ykr[:], ykt[:, 0:1]), icstt)

            # single full-width PSUM tile (all 8 banks): one recycle total
            ps = psum.tile([128, T], F32, tag="ps")
            # PE touches (plain f32 1-col matmuls): absorb the stt DMA,
            # then the ACT drain of the f32r conversions
            mA = nc.tensor.matmul(ps[0:1, 0:1], stt[:, 0:1], stt[:, 0:1],
                                  start=True, stop=True, skip_group_check=True)
            mB = _after(
                nc.tensor.matmul(ps[0:1, 1:2], stt[:, 0:1],
                                 ykr[:, 0:1].bitcast(F32),
                                 start=True, stop=True, skip_group_check=True),
                mA, iyk,
            )

            prev_pe = mB
            prev_sub = None
            SUBW = 1024  # sub chunk: 2 psum banks, overlaps with matmuls
            for r in range(RT):
                yt = io.tile([128, T], F32, tag="yt")
                nc.sync.dma_start(out=yt[:], in_=data[r])
                ot = io.tile([128, T], F32, tag="ot")
                t = tch.tile([128, 1], F32, tag="t")
                vt = nc.vector.tensor_copy(t[:], yt[:, 0:1])  # absorbs yt DMA
                lhsT = sttr[:, r * 128 : (r + 1) * 128]
                if prev_sub is not None:
                    # PE absorber for the ps WAR: DVE writes a bf16 scratch
                    # after the last sub; a standalone bf16 ldweights
                    # (discarded — the next matmul self-loads its weights)
                    # then carries the single DVE wait on PE.
                    bfq = tch.tile([128, 1], mybir.dt.bfloat16, tag="bfq")
                    vb = _after(
                        nc.vector.tensor_copy(bfq[:], ot_prev[:, 0:1]),
                        prev_sub,
                    )
                    prev_pe = _after(
                        nc.tensor.ldweights(bfq[:]), prev_pe, vb
                    )
                subs = []
                for j in range(T // 512):
                    c0 = j * 512
                    if c0 == 0:
                        # filt col 0 = y_0 (plain f32: 1-col f32r matmuls
                        # fail the ISA check); cols 1.. = new shifted by 1
                        prev_pe = _after(
                            nc.tensor.matmul(
                                ps[:, 0:1],
                                stt[:, r * 128 : (r + 1) * 128],
                                ykt[:, 0:1],
                                start=True, stop=True, skip_group_check=True,
                            ),
                            prev_pe,
                        )
                        # 511 cols: f32r needs even sizes -> plain f32 here
                        prev_pe = _after(
                            nc.tensor.matmul(
                                ps[:, 1:512],
                                stt[:, r * 128 : (r + 1) * 128],
                                new[:, 0:511],
                                start=True, stop=True, skip_group_check=True,
                            ),
                            prev_pe,
                        )
                    else:
                        prev_pe = _after(
                            nc.tensor.matmul(
                                ps[:, c0 : c0 + 512],
                                lhsT,
                                newr[:, c0 - 1 : c0 + 511],
                                start=True, stop=True, skip_group_check=True,
                            ),
                            prev_pe,
                        )
                    # after every SUBW columns, subtract that strip (DVE
                    # overlaps with the remaining matmuls); a tiny DVE
                    # touch absorbs the strip's PE wait first
                    if (c0 + 512) % SUBW == 0:
                        s0 = c0 + 512 - SUBW
                        tt = tch.tile([1, 1], F32, tag="tp")
                        vps = _after(
                            nc.vector.tensor_copy(
                                tt[:], ps[0:1, c0 + 511 : c0 + 512]
                            ),
                            vt, prev_pe,
                        )
                        isub2 = _after(
                            nc.vector.tensor_sub(
                                ot[:, s0 : c0 + 512],
                                yt[:, s0 : c0 + 512],
                                ps[:, s0 : c0 + 512],
                            ),
                            vps,
                        )
                        subs.append(isub2)
                prev_sub = subs[-1]
                ot_prev = ot
                nc.sync.dma_start(out=out_o[r], in_=ot[:])
    return nc


# --------------------------------------------------------------------------
# Host orchestration
# --------------------------------------------------------------------------

_P1 = None
_P2 = None


def _programs():
    global _P1, _P2
    if _P1 is None:
        _P1 = _legalize_waits(build_phase1())
        _P2 = _legalize_waits(build_phase2())
    return _P1, _P2


def _selection(errs):
    """Running-min over batch. errs: [B,7] (any common positive scale).
    Returns src[b] (source row for sm[b]) and best_idx[b] (lr idx for pr)."""
    Bn = errs.shape[0]
    loc = np.argmin(errs, axis=1)
    e_loc = errs[np.arange(Bn), loc]
    run_min = np.minimum.accumulate(e_loc)
    prev_min = np.empty_like(run_min)
    prev_min[0] = np.inf
    prev_min[1:] = run_min[:-1]
    improve = e_loc < prev_min
    imp_idx = np.flatnonzero(improve)
    seg = np.searchsorted(imp_idx, np.arange(Bn), side="right") - 1
    src = imp_idx[seg]
    best_idx = loc[src]
    return src, best_idx, imp_idx


def _host_filt(data_rows, sigmas):
    """Reference-faithful sequential filt for a few rows (fallback path)."""
    y = data_rows.astype(np.float32)
    Kn, Tn = y.shape
    s = sigmas.astype(np.float32)[:, None]
    a = np.float32(1.0) - s
    filt = np.empty_like(y)
    state = y[:, 0].copy()
    for tt in range(Tn):
        filt[:, tt] = state
        state = a[:, 0] * state + s[:, 0] * y[:, tt]
    return filt


def kernel(data):
    data = np.ascontiguousarray(np.asarray(data), np.float32)
    assert data.shape == (B, T)
    core_ids = list(range(NCORES))
    p1, p2 = _programs()

    shards = data.reshape(NCORES, RT, 128, T)
    in_maps = [{"data": shards[i]} for i in range(NCORES)]
    res1 = run_bass_kernel_spmd(p1, in_maps, core_ids, trace=TRACE)
    LAST_RESULTS.append(res1)
    errs = np.concatenate(
        [res1.results[i]["errs"].reshape(RPC, NLR) for i in range(NCORES)]
    )
    finals = np.concatenate(
        [res1.results[i]["finals"].reshape(RPC, NLR) for i in range(NCORES)]
    )

    src, best_idx, imp_rows = _selection(errs)
    K = len(imp_rows)

    pr = np.ascontiguousarray(
        np.broadcast_to(
            finals[np.arange(B), best_idx][:, None], (B, PRED_SIZE)
        )
    ).astype(np.float32)

    sig_k = LRS[np.argmin(errs[imp_rows], axis=1)]

    if K > KPAD:  # pathological fallback: pure host reconstruction
        filt_k = _host_filt(data[imp_rows], sig_k)
        row_to_k = {rr: k for k, rr in enumerate(imp_rows)}
        g = np.array([row_to_k[rr] for rr in src])
        out0 = data - filt_k[g]
        return out0, pr

    yk = np.zeros((KPAD, T), np.float32)
    yk[:K] = data[imp_rows]
    ak = np.zeros((KPAD, 1), np.float32)
    sk = np.zeros((KPAD, 1), np.float32)
    ak[:K, 0] = 1.0 - sig_k
    sk[:K, 0] = sig_k
    row_to_k = {rr: k for k, rr in enumerate(imp_rows)}
    g = np.array([row_to_k[rr] for rr in src])
    st_full = np.zeros((KPAD, B), np.float32)
    st_full[g, np.arange(B)] = 1.0

    in_maps2 = [
        {
            "data": shards[i],
            "yk": yk,
            "ak": ak,
            "sk": sk,
            "st": np.ascontiguousarray(st_full[:, i * RPC : (i + 1) * RPC]),
        }
        for i in range(NCORES)
    ]
    res2 = run_bass_kernel_spmd(p2, in_maps2, core_ids, trace=TRACE)
    LAST_RESULTS.append(res2)
    out0 = np.concatenate(
        [res2.results[i]["out"].reshape(RPC, T) for i in range(NCORES)]
    )
    return out0, pr


# revision 29
# speedup vs baseline: 1.0830x; 1.0830x over previous
"""Trainium2 Bass kernel for the Aligator smoothing-filter problem.

Math notes (all derivable from the reference):
  * delta = max-min of each series, and the EMA level always stays inside
    [min, max], so the clip in the reference never binds -> each per-sigma
    filter is the pure linear recurrence new_t = (1-s)*new_{t-1} + s*y_t
    (new_{-1} = y_0), filt_t = new_{t-1} (filt_0 = y_0).
  * The innovation r_t = y_t - new_{t-1} obeys r_{t+1} = (1-s)*r_t + D_t with
    D_t = y_{t+1} - y_t shared by ALL sigmas, r_1 = D_0.  So per sigma we need
    one affine scan over D plus one square-accumulate:
        err_sum = sum_{t>=1} r_t^2     (mean = /T; the t=0 term is exactly 0)
        final   = new_{T-1} = y_{T-1} - (1-s)*r_{T-1}
  * The batch-carryover argmin is a tiny [B,7] running-min scan -> host.
  * sm[b] = filt of the last "improvement" row <= b.  There are only K (~10)
    unique improvement rows; reconstruct their filts with one device scan and
    gather/broadcast them to all 2048 output rows with a one-hot fp32 matmul
    on the TensorEngine, then out = data - sm on the VectorEngine.

Two SPMD NEFFs over 8 cores (batch-sharded 256 rows/core), with the cheap
selection scan on host between them.

Scheduling discipline: this toolchain caps each compute instruction at ONE
sync wait.  Same-engine (drain) waits merge into one, so every instruction
is arranged to have at most one *cross-engine/DMA* dependency that is not
already covered by its engine's vector clock; tiny "touch" ops absorb the
rest ahead of time.
"""

import numpy as np

import concourse.bass as bass
import concourse.mybir as mybir
import concourse.tile as tile
from concourse.bass_utils import run_bass_kernel_spmd

F32 = mybir.dt.float32
AF = mybir.ActivationFunctionType
OP = mybir.AluOpType

LRS = np.array([0.01, 0.08, 0.1, 0.15, 0.2, 0.25, 1.0], dtype=np.float32)
NLR = 7
PRED_SIZE = 64
B, T = 2048, 4096
NCORES = 8
RPC = B // NCORES          # rows per core = 256
RT = RPC // 128            # row-tiles per core = 2
KPAD = 128                 # padded count of improvement rows
CH = 1024                  # scan chunk width (constant-tile width)

# Optionally filled with BassKernelResults by run (test.py reads these).
LAST_RESULTS = []
TRACE = False


def _chunks(n, ch):
    c0 = 0
    while c0 < n:
        yield c0, min(c0 + ch, n)
        c0 += ch


def _legalize_waits(nc, cap_evsem=2):
    """This walrus build caps sync waits at 1 per compute/ctrl instruction
    (2 for EventSemaphore).  Hoist excess waits onto standalone
    EventSemaphore instructions injected just before the offender on the
    same engine queue."""
    import bass_rust

    for fn in nc.m.functions:
        for blk in fn.blocks:
            newinsts = []
            for I in blk.instructions:
                si = I.sync_info
                waits = list(si.on_wait) if (si and si.on_wait) else []
                cap = cap_evsem if isinstance(I, mybir.InstEventSemaphore) else 1
                if len(waits) > cap:
                    keep = waits[-cap:]
                    excess = waits[:-cap]
                    for i in range(0, len(excess), cap_evsem):
                        chunk = excess[i : i + cap_evsem]
                        ev = mybir.InstEventSemaphore(
                            name=f"{I.name}-wsplit-{i}", ins=[], outs=[]
                        )
                        ev.engine = I.engine
                        ev.sync_info = bass_rust.SyncInfo(
                            on_wait=chunk, on_update=[]
                        )
                        newinsts.append(ev)
                    I.sync_info = bass_rust.SyncInfo(
                        on_wait=keep,
                        on_update=list(si.on_update) if si.on_update else [],
                    )
                newinsts.append(I)
            blk.instructions[:] = newinsts
    return nc


def _after(inst, *preds):
    """Pin scheduler ordering (no semaphore): inst must follow preds."""
    from concourse.instruction_name_ordered_set import InstructionNameOrderedSet

    deps = InstructionNameOrderedSet()
    for p in preds:
        deps.add(p.ins.name)
    inst.ins.add_nosync_dependencies_from(deps)
    return inst


# --------------------------------------------------------------------------
# Phase-1 program: errs + finals per (row, sigma)
# --------------------------------------------------------------------------

def build_phase1():
    nc = bass.Bass()
    data = nc.declare_dram_parameter("data", [RT, 128, T], F32, isOutput=False)
    errs_o = nc.declare_dram_parameter("errs", [RT, 128, NLR], F32, isOutput=True)
    finals_o = nc.declare_dram_parameter("finals", [RT, 128, NLR], F32, isOutput=True)

    with tile.TileContext(nc) as tc:
        with (
            tc.tile_pool(name="io", bufs=2) as io,
            tc.tile_pool(name="dl", bufs=2) as dl,
            tc.tile_pool(name="atp", bufs=6) as atp,
            tc.tile_pool(name="rtp", bufs=2) as rtp,
            tc.tile_pool(name="scr", bufs=1) as scr,
            tc.tile_pool(name="tch", bufs=16) as tch,
            tc.tile_pool(name="small", bufs=2) as small,
        ):
            # constant (1-sigma) tiles, written once on DVE (fresh: no waits)
            ats = []
            for si in range(6):
                a_t = atp.tile([128, CH], F32, tag="at")
                nc.vector.memset(a_t[:], float(1.0 - LRS[si]))
                ats.append(a_t)

            erfs = []
            sq_insts = {}  # alloc index -> (erf column AP, square inst)
            HM = T // 2  # 2048; sub0 needs one extra column of y
            for r in range(RT):
                y = io.tile([128, T], F32, tag="y")
                # halved loads so the diffs (and first scans) start early
                nc.sync.dma_start(out=y[:, 0 : HM + 1], in_=data[r, :, 0 : HM + 1])
                nc.sync.dma_start(out=y[:, HM + 1 : T], in_=data[r, :, HM + 1 : T])
                # GpSimd: first differences, half by half (off the DVE)
                dlt = dl.tile([128, T - 1], F32, tag="dlt")
                isub0 = nc.gpsimd.tensor_sub(
                    dlt[:, 0:HM], y[:, 1 : HM + 1], y[:, 0:HM]
                )
                isub = _after(
                    nc.gpsimd.tensor_sub(
                        dlt[:, HM : T - 1], y[:, HM + 1 : T], y[:, HM : T - 1]
                    ),
                    isub0,
                )
                # DVE absorbers: observe the Pool drains per half
                tt = tch.tile([128, 1], F32, tag="t")
                vdlt = _after(nc.vector.tensor_copy(tt[:], dlt[:, 0:1]), isub0)
                tt = tch.tile([128, 1], F32, tag="t")
                vdlt1 = _after(
                    nc.vector.tensor_copy(tt[:], dlt[:, HM : HM + 1]), isub, vdlt
                )
                # ACT: waits on the y DMA only
                ylast = small.tile([128, 1], F32, tag="ylast")
                iyl = nc.scalar.copy(ylast[:], y[:, T - 1 : T])
                erf = small.tile([128, NLR], F32, tag="erf")
                fin = small.tile([128, NLR], F32, tag="fin")
                erfs.append(erf)
                # ACT: absorbs the ylast ACT-drain; sigma=1 final = y_{T-1}
                _after(nc.scalar.copy(fin[:, 6:7], ylast[:]), iyl)
                # sigma=1.0 square early (r-sequence is dlt itself) so the
                # last per-row ACT op is the si=5 square, not this one
                tt = tch.tile([128, 1], F32, tag="t")
                aabs = _after(nc.scalar.copy(tt[:], dlt[:, HM : HM + 1]), isub)
                sq = scr.tile([128, T - 1], F32, tag="sq")
                isq1 = _after(
                    nc.scalar.activation(
                        sq[:], dlt[:], AF.Square, accum_out=erf[:, 6:7]
                    ),
                    aabs,
                )

                for si in range(6):
                    a = float(1.0 - LRS[si])
                    i_alloc = r * 6 + si
                    rt_ = rtp.tile([128, T - 1], F32, tag="rt")
                    vabs = None
                    if i_alloc >= 2:
                        # DVE absorber: wait out the ACT square that last
                        # read the slot this rt_ recycles
                        ecol, prev_sq = sq_insts[i_alloc - 2]
                        tt = tch.tile([128, 1], F32, tag="t")
                        vabs = _after(nc.vector.tensor_copy(tt[:], ecol), prev_sq)
                    # DVE first-touch of the slot (same-engine WAW only now)
                    im = _after(nc.vector.memset(rt_[:, 0:1], 0.0), vdlt)
                    if vabs is not None:
                        _after(im, vabs)
                    # chunked scan: r_{t+1} = a*r_t + D_t (all deps DVE-local;
                    # chunks beyond HM additionally follow the half-1 absorber)
                    for c0, c1 in _chunks(T - 1, CH):
                        init = 0.0 if c0 == 0 else rt_[:, c0 - 1 : c0]
                        isc = nc.vector.tensor_tensor_scan(
                            rt_[:, c0:c1],
                            ats[si][:, 0 : c1 - c0],
                            dlt[:, c0:c1],
                            init,
                            OP.mult,
                            OP.add,
                        )
                        if c1 > HM:
                            _after(isc, vdlt1)
                    # ACT: final first (absorbs rt_'s DVE drain; 1 wait)
                    ifin = nc.scalar.activation(
                        fin[:, si : si + 1],
                        rt_[:, T - 2 : T - 1],
                        AF.Identity,
                        bias=ylast[:],
                        scale=-a,
                    )
                    # ACT: square-accumulate (rt_ covered; 1 wait on sq WAW)
                    sq = scr.tile([128, T - 1], F32, tag="sq")
                    isq = _after(
                        nc.scalar.activation(
                            sq[:], rt_[:], AF.Square,
                            accum_out=erf[:, si : si + 1],
                        ),
                        ifin,
                    )
                    sq_insts[i_alloc] = (erf[:, si : si + 1], isq)

                nc.sync.dma_start(out=errs_o[r], in_=erf[:])
                nc.sync.dma_start(out=finals_o[r], in_=fin[:])
    return nc


# --------------------------------------------------------------------------
# Phase-2 program: reconstruct sm rows and subtract
# --------------------------------------------------------------------------

def build_phase2():
    nc = bass.Bass()
    data = nc.declare_dram_parameter("data", [RT, 128, T], F32, isOutput=False)
    # aux = [ ab (CH cols: a_k bcast) | y0 (1 col) | d1 = s_k*y_k (T cols) ]
    aux = nc.declare_dram_parameter("aux", [KPAD, CH + 1 + T], F32, isOutput=False)
    st = nc.declare_dram_parameter("st", [KPAD, RPC], F32, isOutput=False)
    out_o = nc.declare_dram_parameter("out", [RT, 128, T], F32, isOutput=True)

    SPLIT = CH + 1 + CH  # aux part 1 covers ab, y0 and d1 chunk 0
    D0 = CH + 1          # d1 column offset inside aux
    CSPLIT = 2560        # newr conversion halves (matmul chunk 5 boundary)

    with tile.TileContext(nc) as tc:
        with (
            tc.tile_pool(name="cons", bufs=1) as cons,
            tc.tile_pool(name="tch", bufs=24) as tch,
            tc.tile_pool(name="io", bufs=2) as io,
            tc.tile_pool(name="psum", bufs=1, space="PSUM") as psum,
        ):
            F32R = mybir.dt.float32r
            auxt = cons.tile([KPAD, CH + 1 + T], F32)
            nc.sync.dma_start(out=auxt[:, 0:SPLIT], in_=aux[:, 0:SPLIT])
            nc.sync.dma_start(out=auxt[:, SPLIT:], in_=aux[:, SPLIT:])
            stt = cons.tile([KPAD, RPC], F32)
            nc.sync.dma_start(out=stt[:], in_=st[:])

            # ACT: f32r one-hot matrix (waits only the st DMA)
            sttr = cons.tile([KPAD, RPC], F32R)
            icstt = nc.scalar.copy(sttr[:], stt[:])

            # DVE: scans over host-built d1/ab; part-1 touch, then chunk 0
            t = tch.tile([KPAD, 1], F32, tag="t")
            v1 = nc.vector.tensor_copy(t[:], auxt[:, 0:1])
            new = cons.tile([KPAD, T], F32)
            sc = {}
            prev = _after(
                nc.vector.tensor_tensor_scan(
                    new[:, 0:CH], auxt[:, 0:CH], auxt[:, D0 : D0 + CH],
                    auxt[:, CH : CH + 1], OP.mult, OP.add,
                ),
                v1,
            )
            sc[0] = prev
            t = tch.tile([KPAD, 1], F32, tag="t")
            v2 = _after(nc.vector.tensor_copy(t[:], auxt[:, SPLIT : SPLIT + 1]), v1)
            for ci, (c0, c1) in enumerate(_chunks(T, CH)):
                if c0 == 0:
                    continue
                prev = _after(
                    nc.vector.tensor_tensor_scan(
                        new[:, c0:c1], auxt[:, 0 : c1 - c0],
                        auxt[:, D0 + c0 : D0 + c1],
                        new[:, c0 - 1 : c0], OP.mult, OP.add,
                    ),
                    prev, v2,
                )
                sc[ci] = prev

            # ACT: f32r conversion in two halves, right behind the scans
            newr = cons.tile([KPAD, T], F32R)
            icnA = _after(nc.scalar.copy(newr[:, 0:CSPLIT], new[:, 0:CSPLIT]),
                          icstt, sc[2])
            icnB = _after(nc.scalar.copy(newr[:, CSPLIT:T], new[:, CSPLIT:T]),
                          icnA, sc[3])

            # single full-width PSUM tile (all 8 banks): one recycle total
            ps = psum.tile([128, T], F32, tag="ps")
            # PE touches: st DMA, aux DMA, then the ACT conversions
            mA = nc.tensor.matmul(ps[0:1, 0:1], stt[:, 0:1], stt[:, 0:1],
                                  start=True, stop=True, skip_group_check=True)
            mA2 = _after(
                nc.tensor.matmul(ps[0:1, 2:3], stt[:, 0:1],
                                 auxt[:, CH : CH + 1],
                                 start=True, stop=True, skip_group_check=True),
                mA,
            )
            mB = _after(
                nc.tensor.matmul(ps[0:1, 1:2], stt[:, 0:1],
                                 newr[:, 0:1].bitcast(F32),
                                 start=True, stop=True, skip_group_check=True),
                mA2, icnA,
            )
            # convB absorber: virgin bank-7 cell, before matmul chunk 5
            mC = _after(
                nc.tensor.matmul(ps[0:1, 4094:4095], stt[:, 0:1],
                                 newr[:, T - 1 : T].bitcast(F32),
                                 start=True, stop=True, skip_group_check=True),
                mB, icnB,
            )

            prev_pe = mB
            prev_sub = None
            SUBW = 1024  # sub chunk: 2 psum banks, overlaps with matmuls
            for r in range(RT):
                yt = io.tile([128, T], F32, tag="yt")
                nc.sync.dma_start(out=yt[:], in_=data[r])
                ot = io.tile([128, T], F32, tag="ot")
                t = tch.tile([128, 1], F32, tag="t")
                vt = nc.vector.tensor_copy(t[:], yt[:, 0:1])  # absorbs yt DMA
                lhsT = sttr[:, r * 128 : (r + 1) * 128]
                if prev_sub is not None:
                    bfq = tch.tile([128, 1], mybir.dt.bfloat16, tag="bfq")
                    vb = _after(
                        nc.vector.tensor_copy(bfq[:], ot_prev[:, 0:1]),
                        prev_sub,
                    )
                    prev_pe = _after(nc.tensor.ldweights(bfq[:]), prev_pe, vb)
                subs = []
                for j in range(T // 512):
                    c0 = j * 512
                    if c0 == 0:
                        prev_pe = _after(
                            nc.tensor.matmul(
                                ps[:, 0:1],
                                stt[:, r * 128 : (r + 1) * 128],
                                auxt[:, CH : CH + 1],
                                start=True, stop=True, skip_group_check=True,
                            ),
                            prev_pe,
                        )
                        # 511 cols via the (already f32r-rounded) newr bits
                        prev_pe = _after(
                            nc.tensor.matmul(
                                ps[:, 1:512],
                                stt[:, r * 128 : (r + 1) * 128],
                                newr[:, 0:511].bitcast(F32),
                                start=True, stop=True, skip_group_check=True,
                            ),
                            prev_pe,
                        )
                    else:
                        if c0 == 2560 and r == 0:
                            prev_pe = _after(mC, prev_pe)
                        prev_pe = _after(
                            nc.tensor.matmul(
                                ps[:, c0 : c0 + 512],
                                lhsT,
                                newr[:, c0 - 1 : c0 + 511],
                                start=True, stop=True, skip_group_check=True,
                            ),
                            prev_pe,
                        )
                    if (c0 + 512) % SUBW == 0:
                        s0 = c0 + 512 - SUBW
                        tt = tch.tile([1, 1], F32, tag="tp")
                        vps = _after(
                            nc.vector.tensor_copy(
                                tt[:], ps[0:1, c0 + 511 : c0 + 512]
                            ),
                            vt, prev_pe,
                        )
                        isub2 = _after(
                            nc.vector.tensor_sub(
                                ot[:, s0 : c0 + 512],
                                yt[:, s0 : c0 + 512],
                                ps[:, s0 : c0 + 512],
                            ),
                            vps,
                        )
                        subs.append(isub2)
                prev_sub = subs[-1]
                ot_prev = ot
                nc.sync.dma_start(out=out_o[r], in_=ot[:])
    return nc


# --------------------------------------------------------------------------
# Host orchestration
# --------------------------------------------------------------------------

_P1 = None
_P2 = None


def _programs():
    global _P1, _P2
    if _P1 is None:
        _P1 = _legalize_waits(build_phase1())
        _P2 = _legalize_waits(build_phase2())
    return _P1, _P2


def _selection(errs):
    """Running-min over batch. errs: [B,7] (any common positive scale).
    Returns src[b] (source row for sm[b]) and best_idx[b] (lr idx for pr)."""
    Bn = errs.shape[0]
    loc = np.argmin(errs, axis=1)
    e_loc = errs[np.arange(Bn), loc]
    run_min = np.minimum.accumulate(e_loc)
    prev_min = np.empty_like(run_min)
    prev_min[0] = np.inf
    prev_min[1:] = run_min[:-1]
    improve = e_loc < prev_min
    imp_idx = np.flatnonzero(improve)
    seg = np.searchsorted(imp_idx, np.arange(Bn), side="right") - 1
    src = imp_idx[seg]
    best_idx = loc[src]
    return src, best_idx, imp_idx


def _host_filt(data_rows, sigmas):
    """Reference-faithful sequential filt for a few rows (fallback path)."""
    y = data_rows.astype(np.float32)
    Kn, Tn = y.shape
    s = sigmas.astype(np.float32)[:, None]
    a = np.float32(1.0) - s
    filt = np.empty_like(y)
    state = y[:, 0].copy()
    for tt in range(Tn):
        filt[:, tt] = state
        state = a[:, 0] * state + s[:, 0] * y[:, tt]
    return filt


def kernel(data):
    data = np.ascontiguousarray(np.asarray(data), np.float32)
    assert data.shape == (B, T)
    core_ids = list(range(NCORES))
    p1, p2 = _programs()

    shards = data.reshape(NCORES, RT, 128, T)
    in_maps = [{"data": shards[i]} for i in range(NCORES)]
    res1 = run_bass_kernel_spmd(p1, in_maps, core_ids, trace=TRACE)
    LAST_RESULTS.append(res1)
    errs = np.concatenate(
        [res1.results[i]["errs"].reshape(RPC, NLR) for i in range(NCORES)]
    )
    finals = np.concatenate(
        [res1.results[i]["finals"].reshape(RPC, NLR) for i in range(NCORES)]
    )

    src, best_idx, imp_rows = _selection(errs)
    K = len(imp_rows)

    pr = np.ascontiguousarray(
        np.broadcast_to(
            finals[np.arange(B), best_idx][:, None], (B, PRED_SIZE)
        )
    ).astype(np.float32)

    sig_k = LRS[np.argmin(errs[imp_rows], axis=1)]

    if K > KPAD:  # pathological fallback: pure host reconstruction
        filt_k = _host_filt(data[imp_rows], sig_k)
        row_to_k = {rr: k for k, rr in enumerate(imp_rows)}
        g = np.array([row_to_k[rr] for rr in src])
        out0 = data - filt_k[g]
        return out0, pr

    yk = data[imp_rows]
    # aux = [ ab (a_k bcast over CH) | y0 | d1 = s_k * y_k ]
    aux = np.zeros((KPAD, CH + 1 + T), np.float32)
    aux[:K, 0:CH] = (1.0 - sig_k)[:, None]
    aux[:K, CH] = yk[:, 0]
    aux[:K, CH + 1 :] = sig_k[:, None] * yk
    row_to_k = {rr: k for k, rr in enumerate(imp_rows)}
    g = np.array([row_to_k[rr] for rr in src])
    st_full = np.zeros((KPAD, B), np.float32)
    st_full[g, np.arange(B)] = 1.0

    in_maps2 = [
        {
            "data": shards[i],
            "aux": aux,
            "st": np.ascontiguousarray(st_full[:, i * RPC : (i + 1) * RPC]),
        }
        for i in range(NCORES)
    ]
    res2 = run_bass_kernel_spmd(p2, in_maps2, core_ids, trace=TRACE)
    LAST_RESULTS.append(res2)
    out0 = np.concatenate(
        [res2.results[i]["out"].reshape(RPC, T) for i in range(NCORES)]
    )
    return out0, pr
